# revision 11
# baseline (speedup 1.0000x reference)
"""Trainium2 Bass kernel for the CMPO2/GTN MPS-chain contraction.

Computation (see harness reference): for each sample s,
    v0  = psi_first^T x[s,0]                                  [D]
    v_{i+1}[e] = sum_{d,p} v_i[d] psi_mid[i][d,e,p] x[s,1+i,p]   (62 steps)
    out_vec[s] = sum_{d,p} v[d] psi_last[d,p,:] x[s,63,p]     [O]
    out[s] = c * out_vec[s]   with c the (batch-independent) phi-chain scalar.

Strategy: data-parallel over batch across 8 cores (1024 samples/core),
MPS parameters replicated.  Per middle step the contraction is mapped as
    u[s,(p,d)] = v[s,d] * x[s,p]        (outer product, fp16, p-major rows)
    v_new      = u @ A_flat             (PE matmul, K=2048 in 16 chunks of 128)
The per-sample v broadcast is obtained for free: the A stationaries are
column-duplicated so each accumulation chain outputs [vT; vT] on all 128
PSUM partitions, and the x-side partition broadcast (which is input data,
not dependent on v) is precomputed on the host and streamed from HBM by
the otherwise-idle DMA engines.  The vector engine does the outer products
(fp16 2x mode); the scalar engine only evacuates the small [vT; vT].
fp16 overflow is prevented by folding static power-of-2 scales (derived
from a host-side subsample) into the A weights; the inverse scale is
folded into the phi-chain scalar, computed on-device in fp32.
"""

import numpy as np

N_CORES = 8
B, Q, P, D, L, O = 8192, 64, 32, 64, 64, 10
BL = B // N_CORES          # batch per core
TN = 512                   # matmul free-dim tile (one PSUM bank of fp32)
NT = BL // TN              # N tiles per batch shard
NCH = (D * P) // 128       # 16 K-chunks of 128 over (p,d)
NG = 8                     # chunk pairs (2 chunks each) for paired DVE muls
NMID = L - 2               # 62 middle sites
SH_LAST = 6                # 2^SH_LAST folded into psi_last (fp16 subnormal avoidance)
VBAND = 16.0               # target |v| band for the scale schedule

# global row r in 0..2047 of u/(A rows): p = 2*(r//128) + (r%128)//64 ; d = r%64
_P_IDX = np.repeat(np.arange(P), D)          # [2048]
_D_IDX = np.tile(np.arange(D), P)            # [2048]

_cached = {}


def _ensure_path():
    import sys
    for p in ("/opt/trn_rl_repo", "/root/.axon_site/_ro/trn_rl_repo"):
        try:
            import concourse  # noqa: F401
            return
        except Exception:
            if p not in sys.path:
                sys.path.insert(0, p)
    import concourse  # noqa: F401


def _build_program():
    """Build + compile the Bass/Tile program (shared by all 8 cores)."""
    _ensure_path()
    from concourse import bacc, tile, mybir

    dt = mybir.dt
    nc = bacc.Bacc(
        "TRN2",
        target_bir_lowering=False,
        debug=False,
        enable_asserts=False,
        num_devices=N_CORES,
    )

    a_d = nc.dram_tensor("a_w", [NMID, 128, NCH * 128], dt.float16, kind="ExternalInput").ap()
    xb_d = nc.dram_tensor("xb", [NMID + 1, NG, 128, 2 * BL], dt.float16, kind="ExternalInput").ap()
    x0_d = nc.dram_tensor("x0", [P, BL], dt.float16, kind="ExternalInput").ap()
    pf_d = nc.dram_tensor("pf", [P, 128], dt.float16, kind="ExternalInput").ap()
    pl_d = nc.dram_tensor("pl", [128, NCH * O], dt.float16, kind="ExternalInput").ap()
    pm_d = nc.dram_tensor("phim", [D, NMID * D], dt.float32, kind="ExternalInput").ap()
    w0_d = nc.dram_tensor("w0", [D, 1], dt.float32, kind="ExternalInput").ap()
    plc_d = nc.dram_tensor("phil", [D, 1], dt.float32, kind="ExternalInput").ap()
    out_d = nc.dram_tensor("out", [O, BL], dt.float32, kind="ExternalOutput").ap()

    with tile.TileContext(nc) as tc:
        with tc.tile_pool(name="const", bufs=1) as cpool, \
             tc.tile_pool(name="aw", bufs=3) as apool, \
             tc.tile_pool(name="xbp", bufs=20) as xbpool, \
             tc.tile_pool(name="vrp", bufs=2) as vrpool, \
             tc.tile_pool(name="up", bufs=10) as upool, \
             tc.tile_pool(name="misc", bufs=1) as mpool, \
             tc.tile_pool(name="wp", bufs=2) as wpool, \
             tc.tile_pool(name="pvp", bufs=4, space="PSUM") as pvpool, \
             tc.tile_pool(name="phpp", bufs=3, space="PSUM") as phpool:

            # --- constants / per-core inputs resident in SBUF ---
            pf_sb = cpool.tile([P, 128], dt.float16, name="pf_sb")
            nc.sync.dma_start(out=pf_sb, in_=pf_d)
            pl_sb = cpool.tile([128, NCH * O], dt.float16, name="pl_sb")
            nc.sync.dma_start(out=pl_sb, in_=pl_d)
            pm_sb = cpool.tile([D, NMID * D], dt.float32, name="pm_sb")
            nc.sync.dma_start(out=pm_sb, in_=pm_d)
            plc_sb = cpool.tile([D, 1], dt.float32, name="plc_sb")
            nc.sync.dma_start(out=plc_sb, in_=plc_d)
            x0_sb = cpool.tile([P, BL], dt.float16, name="x0_sb")
            nc.sync.dma_start(out=x0_sb, in_=x0_d)

            w_cur = wpool.tile([D, 1], dt.float32, name="wv")
            nc.sync.dma_start(out=w_cur, in_=w0_d)

            # --- v0 = [psi_first^T | psi_first^T] @ x0 -> [v0 ; v0] ---
            pv_cur = []
            for t in range(NT):
                pv = pvpool.tile([128, TN], dt.float32, name="pv")
                nc.tensor.matmul(out=pv, lhsT=pf_sb,
                                 rhs=x0_sb[:, t * TN:(t + 1) * TN],
                                 start=True, stop=True)
                pv_cur.append(pv)

            po = None
            for i in range(NMID + 1):
                last = (i == NMID)
                # evacuate [vT; vT] into v2[:, 0:BL]; one fast DVE copy fills
                # the second copy so pair-muls read a [128, 2*BL] operand.
                v2 = vrpool.tile([128, 2 * BL], dt.float16, name="v2")
                for t in range(NT):
                    nc.scalar.copy(out=v2[:, t * TN:(t + 1) * TN], in_=pv_cur[t])
                nc.vector.tensor_copy(v2[:, BL:2 * BL], v2[:, 0:BL])

                # x-side broadcast tiles from HBM (prefetched by Tile)
                xbs = []
                for g in range(NG):
                    xb_sb = xbpool.tile([128, 2 * BL], dt.float16, name="xb_sb")
                    nc.sync.dma_start(out=xb_sb, in_=xb_d[i, g])
                    xbs.append(xb_sb)

                if not last:
                    a_sb = apool.tile([128, NCH * 128], dt.float16, name="a_sb")
                    nc.sync.dma_start(out=a_sb, in_=a_d[i])
                    pv_nxt = []
                    for t in range(NT):
                        pv = pvpool.tile([128, TN], dt.float32, name="pv")
                        pv_nxt.append(pv)
                else:
                    po = []
                    for t in range(NT):
                        p_o = pvpool.tile([O, TN], dt.float32, name="pv")
                        po.append(p_o)

                # outer products: pair 0 as 4 fine-grained per-Ntile muls
                # (reading v2[:, t-tile] only, so the PE restarts fast);
                # pairs 1..7 as one [128, 2048] mul each.
                us = []
                for g in range(NG):
                    u2 = upool.tile([128, 2 * BL], dt.float16, name="u2")
                    us.append(u2)
                for q in range(2):
                    for t in range(NT):
                        sl = slice(q * BL + t * TN, q * BL + (t + 1) * TN)
                        nc.vector.tensor_mul(us[0][:, sl], v2[:, t * TN:(t + 1) * TN],
                                             xbs[0][:, sl])
                for g in range(1, NG):
                    nc.vector.tensor_mul(us[g], v2, xbs[g])

                # Tiny matmuls keyed off mid-phase DVE outputs keep the PE's
                # activity monitor warm through the outer-product phase, so
                # the main matmuls don't restart at the throttled clock.
                for gg in (2, 4, 6):
                    warm = phpool.tile([1, 1], dt.float32, name="php")
                    nc.tensor.matmul(out=warm, lhsT=us[gg][:, 0:1],
                                     rhs=us[gg][:, 1:2], start=True, stop=True)

                lhs_sb = pl_sb if last else a_sb
                lhs_w = O if last else 128
                out_ps = po if last else pv_nxt
                for c in range(NCH):
                    g, q = c // 2, c % 2
                    for t in range(NT):
                        sl = slice(q * BL + t * TN, q * BL + (t + 1) * TN)
                        nc.tensor.matmul(
                            out=out_ps[t],
                            lhsT=lhs_sb[:, c * lhs_w:(c + 1) * lhs_w],
                            rhs=us[g][:, sl],
                            start=(c == 0), stop=(c == NCH - 1))
                if not last:
                    # phi chain matvec, interleaved (PE fp32, tiny)
                    php = phpool.tile([D, 1], dt.float32, name="php")
                    nc.tensor.matmul(out=php, lhsT=pm_sb[:, i * D:(i + 1) * D],
                                     rhs=w_cur, start=True, stop=True)
                    w_nxt = wpool.tile([D, 1], dt.float32, name="wv")
                    nc.scalar.copy(out=w_nxt, in_=php)
                    w_cur = w_nxt
                    pv_cur = pv_nxt

            # --- c = w^T phi_last' ; broadcast to O partitions; scale output ---
            cps = phpool.tile([1, 1], dt.float32, name="php")
            nc.tensor.matmul(out=cps, lhsT=plc_sb, rhs=w_cur, start=True, stop=True)
            c_sb = mpool.tile([1, 1], dt.float32, name="c_sb")
            nc.scalar.copy(out=c_sb, in_=cps)
            c10_sb = mpool.tile([O, 1], dt.float32, name="c10_sb")
            nc.gpsimd.partition_broadcast(c10_sb, c_sb)

            out_sb = mpool.tile([O, BL], dt.float32, name="out_sb")
            for t in range(NT):
                nc.scalar.mul(out=out_sb[:, t * TN:(t + 1) * TN], in_=po[t], mul=c10_sb)
            nc.sync.dma_start(out=out_d, in_=out_sb)

    nc.compile()
    return nc


def _scale_schedule(x, psi_first, psi_mid, nsub=128):
    """Static per-step power-of-2 downscales keeping |v| in a small band."""
    xs = np.asarray(x[:nsub], np.float32)
    v = xs[:, 0] @ np.asarray(psi_first, np.float32).T
    ks = []
    for i in range(NMID):
        A = np.asarray(psi_mid[i], np.float32)            # [d, e, p]
        xi = xs[:, 1 + i]                                  # [s, p]
        u = np.einsum('sd,sp->sdp', v, xi).reshape(nsub, D * P)
        v = u @ A.transpose(0, 2, 1).reshape(D * P, D)
        vm = float(np.abs(v).max())
        k = 0
        while vm * 2.0 ** (-k) > VBAND:
            k += 1
        ks.append(k)
        v = v * 2.0 ** (-k)
    return ks


def kernel(x, psi_first, psi_mid, psi_last, phi_first, phi_mid, phi_last):
    _ensure_path()
    from concourse import bass_utils

    f16 = np.float16
    x = np.asarray(x, np.float32)
    psi_first = np.asarray(psi_first, np.float32)
    psi_mid = np.asarray(psi_mid, np.float32)
    psi_last = np.asarray(psi_last, np.float32)
    phi_first = np.asarray(phi_first, np.float32)
    phi_mid = np.asarray(phi_mid, np.float32)
    phi_last = np.asarray(phi_last, np.float32)

    if "nc" not in _cached:
        _cached["nc"] = _build_program()
    nc = _cached["nc"]

    ks = _scale_schedule(x, psi_first, psi_mid)

    # --- shared weight-side arrays (p-major rows, duplicated columns) ---
    scales = (2.0 ** -np.asarray(ks, np.float64)).astype(np.float32)
    # A2[i, r, e] = psi_mid[i, d(r), e, p(r)] * s_i  -> [62, 2048, 64]
    A2 = psi_mid.transpose(0, 1, 3, 2)[:, _D_IDX, _P_IDX, :]        # [62, 2048, 64]
    A2 = A2 * scales[:, None, None]
    A2c = A2.reshape(NMID, NCH, 128, D)
    A2dup = np.concatenate([A2c, A2c], axis=3)                      # [62, 16, 128, 128]
    a_host = np.ascontiguousarray(
        A2dup.transpose(0, 2, 1, 3).reshape(NMID, 128, NCH * 128)
    ).astype(f16)

    pf_host = np.concatenate([psi_first.T, psi_first.T], axis=1).astype(f16)  # [32, 128]

    # pl2[r, o] = psi_last[d(r), p(r), o] * 2^SH -> chunked [128, 16*O]
    pl2 = (psi_last * (2.0 ** SH_LAST))[_D_IDX, _P_IDX, :]          # [2048, O]
    pl_host = np.ascontiguousarray(
        pl2.reshape(NCH, 128, O).transpose(1, 0, 2).reshape(128, NCH * O)
    ).astype(f16)

    phiM = phi_mid[np.arange(NMID), :, :, np.arange(1, NMID + 1)]   # [62, e, f]
    pm_host = np.ascontiguousarray(phiM.transpose(1, 0, 2).reshape(D, NMID * D)).astype(np.float32)
    w0_host = np.ascontiguousarray(phi_first[:, 0:1]).astype(np.float32)
    plc_host = np.ascontiguousarray(
        phi_last[:, Q - 1:Q] * (2.0 ** (sum(ks) - SH_LAST))
    ).astype(np.float32)

    # --- per-core batch shards: x-side broadcast [63, NG, 128, 4*BL] ---
    xt = x.transpose(1, 2, 0).astype(f16)         # [Q, P, B]
    x0_all = xt[0]                                # [P, B]
    # chunk c rows r(128): p = 2c + r//64 ; group tile [128, 4*BL]:
    #   xb[i, g, r, q*BL + s] = xt[1+i, 2*(4g+q) + r//64, s]
    in_maps = []
    for ci in range(N_CORES):
        sl = slice(ci * BL, (ci + 1) * BL)
        xs = np.ascontiguousarray(xt[1:, :, sl])            # [63, P, BL]
        xb = xs[:, np.repeat(np.arange(P), D), :]           # [63, 2048, BL]
        xb = xb.reshape(NMID + 1, NG, 2, 128, BL).transpose(0, 1, 3, 2, 4)
        xb = np.ascontiguousarray(xb.reshape(NMID + 1, NG, 128, 2 * BL))
        in_maps.append({
            "a_w": a_host,
            "xb": xb,
            "x0": np.ascontiguousarray(x0_all[:, sl]),
            "pf": pf_host,
            "pl": pl_host,
            "phim": pm_host,
            "w0": w0_host,
            "phil": plc_host,
        })

    res = bass_utils.run_bass_kernel_spmd(nc, in_maps, core_ids=list(range(N_CORES)))
    _cached["in_maps"] = in_maps

    out = np.empty((B, O), np.float32)
    for ci in range(N_CORES):
        out[ci * BL:(ci + 1) * BL, :] = res.results[ci]["out"].T
    return out


# revision 14
# speedup vs baseline: 1.3747x; 1.3747x over previous
"""Trainium2 Bass kernel for the CMPO2/GTN MPS-chain contraction.

Computation (see harness reference): for each sample s,
    v0  = psi_first^T x[s,0]                                  [D]
    v_{i+1}[e] = sum_{d,p} v_i[d] psi_mid[i][d,e,p] x[s,1+i,p]   (62 steps)
    out_vec[s] = sum_{d,p} v[d] psi_last[d,p,:] x[s,63,p]     [O]
    out[s] = c * out_vec[s]   with c the (batch-independent) phi-chain scalar.

Strategy: data-parallel over batch across 8 cores (1024 samples/core),
MPS parameters replicated.  Per middle step the contraction is mapped as
    u[s,(p,d)] = v[s,d] * x[s,p]        (outer product, fp16, p-major rows)
    v_new      = u @ A_flat             (PE matmul, K=2048 in 16 chunks of 128)
The per-sample v broadcast is obtained for free: the A stationaries are
column-duplicated so each accumulation chain outputs [vT; vT] on all 128
PSUM partitions, and the x-side partition broadcast (which is input data,
not dependent on v) is precomputed on the host and streamed from HBM by
the otherwise-idle DMA engines.  The vector engine does the outer products
(fp16 2x mode); the scalar engine only evacuates the small [vT; vT].
fp16 overflow is prevented by folding static power-of-2 scales (derived
from a host-side subsample) into the A weights; the inverse scale is
folded into the phi-chain scalar, computed on-device in fp32.
"""

import numpy as np

N_CORES = 8
B, Q, P, D, L, O = 8192, 64, 32, 64, 64, 10
BL = B // N_CORES          # batch per core
TN = 512                   # matmul free-dim tile (one PSUM bank of fp32)
NT = BL // TN              # N tiles per batch shard
NCH = (D * P) // 128       # 16 K-chunks of 128 over (p,d)
NG = 8                     # chunk pairs (2 chunks each) for paired DVE muls
NMID = L - 2               # 62 middle sites
SH_LAST = 6                # 2^SH_LAST folded into psi_last (fp16 subnormal avoidance)
VBAND = 16.0               # target |v| band for the scale schedule

# global row r in 0..2047 of u/(A rows): p = 2*(r//128) + (r%128)//64 ; d = r%64
_P_IDX = np.repeat(np.arange(P), D)          # [2048]
_D_IDX = np.tile(np.arange(D), P)            # [2048]

_cached = {}


def _ensure_path():
    import sys
    for p in ("/opt/trn_rl_repo", "/root/.axon_site/_ro/trn_rl_repo"):
        try:
            import concourse  # noqa: F401
            return
        except Exception:
            if p not in sys.path:
                sys.path.insert(0, p)
    import concourse  # noqa: F401


def _build_program():
    """Build + compile the Bass/Tile program (shared by all 8 cores)."""
    _ensure_path()
    from concourse import bacc, tile, mybir

    dt = mybir.dt
    nc = bacc.Bacc(
        "TRN2",
        target_bir_lowering=False,
        debug=False,
        enable_asserts=False,
        num_devices=N_CORES,
    )

    a_d = nc.dram_tensor("a_w", [NMID, 128, NCH * 128], dt.float16, kind="ExternalInput").ap()
    xb_d = nc.dram_tensor("xb", [NMID + 1, NG, 128, 2 * BL], dt.float16, kind="ExternalInput").ap()
    x0_d = nc.dram_tensor("x0", [P, BL], dt.float16, kind="ExternalInput").ap()
    pf_d = nc.dram_tensor("pf", [P, 128], dt.float16, kind="ExternalInput").ap()
    pl_d = nc.dram_tensor("pl", [128, NCH * O], dt.float16, kind="ExternalInput").ap()
    pm_d = nc.dram_tensor("phim", [D, NMID * D], dt.float32, kind="ExternalInput").ap()
    w0_d = nc.dram_tensor("w0", [D, 1], dt.float32, kind="ExternalInput").ap()
    plc_d = nc.dram_tensor("phil", [D, 1], dt.float32, kind="ExternalInput").ap()
    out_d = nc.dram_tensor("out", [O, BL], dt.float32, kind="ExternalOutput").ap()

    with tile.TileContext(nc) as tc:
        with tc.tile_pool(name="const", bufs=1) as cpool, \
             tc.tile_pool(name="aw", bufs=3) as apool, \
             tc.tile_pool(name="xbp", bufs=20) as xbpool, \
             tc.tile_pool(name="vrp", bufs=2) as vrpool, \
             tc.tile_pool(name="up", bufs=10) as upool, \
             tc.tile_pool(name="misc", bufs=1) as mpool, \
             tc.tile_pool(name="wp", bufs=2) as wpool, \
             tc.tile_pool(name="pvp", bufs=4, space="PSUM") as pvpool, \
             tc.tile_pool(name="phpp", bufs=2, space="PSUM") as phpool:

            # --- constants / per-core inputs resident in SBUF ---
            pf_sb = cpool.tile([P, 128], dt.float16, name="pf_sb")
            nc.sync.dma_start(out=pf_sb, in_=pf_d)
            pl_sb = cpool.tile([128, NCH * O], dt.float16, name="pl_sb")
            nc.sync.dma_start(out=pl_sb, in_=pl_d)
            pm_sb = cpool.tile([D, NMID * D], dt.float32, name="pm_sb")
            nc.sync.dma_start(out=pm_sb, in_=pm_d)
            plc_sb = cpool.tile([D, 1], dt.float32, name="plc_sb")
            nc.sync.dma_start(out=plc_sb, in_=plc_d)
            x0_sb = cpool.tile([P, BL], dt.float16, name="x0_sb")
            nc.sync.dma_start(out=x0_sb, in_=x0_d)

            w_cur = wpool.tile([D, 1], dt.float32, name="wv")
            nc.sync.dma_start(out=w_cur, in_=w0_d)

            # --- v0 = [psi_first^T | psi_first^T] @ x0 -> [v0 ; v0] ---
            pv_cur = []
            for t in range(NT):
                pv = pvpool.tile([128, TN], dt.float32, name="pv")
                nc.tensor.matmul(out=pv, lhsT=pf_sb,
                                 rhs=x0_sb[:, t * TN:(t + 1) * TN],
                                 start=True, stop=True)
                pv_cur.append(pv)

            po = None
            for i in range(NMID + 1):
                last = (i == NMID)
                # evacuate [vT; vT] into v2[:, 0:BL] per N-tile half, each
                # followed by its own dup copy, so each half's outer products
                # and matmuls can proceed while the other half is still in
                # flight (keeps the PE warm and busy across the step chain).
                v2 = vrpool.tile([128, 2 * BL], dt.float16, name="v2")
                for t in range(NT):
                    nc.scalar.copy(out=v2[:, t * TN:(t + 1) * TN], in_=pv_cur[t])
                    nc.vector.tensor_copy(v2[:, BL + t * TN:BL + (t + 1) * TN],
                                          v2[:, t * TN:(t + 1) * TN])

                # x-side broadcast tiles from HBM (prefetched by Tile)
                xbs = []
                for g in range(NG):
                    xb_sb = xbpool.tile([128, 2 * BL], dt.float16, name="xb_sb")
                    nc.sync.dma_start(out=xb_sb, in_=xb_d[i, g])
                    xbs.append(xb_sb)

                if not last:
                    a_sb = apool.tile([128, NCH * 128], dt.float16, name="a_sb")
                    nc.sync.dma_start(out=a_sb, in_=a_d[i])
                    pv_nxt = []
                    for t in range(NT):
                        pv = pvpool.tile([128, TN], dt.float32, name="pv")
                        pv_nxt.append(pv)
                else:
                    po = []
                    for t in range(NT):
                        p_o = pvpool.tile([O, TN], dt.float32, name="pv")
                        po.append(p_o)

                # outer products and matmuls, emitted per N-tile half so the
                # two halves software-pipeline: while the DVE produces half
                # t1's u tiles, the PE consumes half t0's.  Columns of
                # v2/xb/u tiles are laid out (q, t, s) with q the chunk
                # within the pair, so the t-half of a pair is a strided view.
                us = []
                for g in range(NG):
                    u2 = upool.tile([128, 2 * BL], dt.float16, name="u2")
                    us.append(u2)
                lhs_sb = pl_sb if last else a_sb
                lhs_w = O if last else 128
                out_ps = po if last else pv_nxt
                for t in range(NT):
                    for q in range(2):
                        sl = slice(q * BL + t * TN, q * BL + (t + 1) * TN)
                        nc.vector.tensor_mul(us[0][:, sl], v2[:, t * TN:(t + 1) * TN],
                                             xbs[0][:, sl])
                    for g in range(1, NG):
                        v2t = v2.rearrange("p (q t s) -> p q t s", q=2, t=NT, s=TN)[:, :, t, :]
                        xbt = xbs[g].rearrange("p (q t s) -> p q t s", q=2, t=NT, s=TN)[:, :, t, :]
                        ut = us[g].rearrange("p (q t s) -> p q t s", q=2, t=NT, s=TN)[:, :, t, :]
                        nc.vector.tensor_mul(ut, v2t, xbt)
                    for c in range(NCH):
                        g, q = c // 2, c % 2
                        sl = slice(q * BL + t * TN, q * BL + (t + 1) * TN)
                        nc.tensor.matmul(
                            out=out_ps[t],
                            lhsT=lhs_sb[:, c * lhs_w:(c + 1) * lhs_w],
                            rhs=us[g][:, sl],
                            start=(c == 0), stop=(c == NCH - 1))
                if not last:
                    # phi chain matvec, interleaved (PE fp32, tiny)
                    php = phpool.tile([D, 1], dt.float32, name="php")
                    nc.tensor.matmul(out=php, lhsT=pm_sb[:, i * D:(i + 1) * D],
                                     rhs=w_cur, start=True, stop=True)
                    w_nxt = wpool.tile([D, 1], dt.float32, name="wv")
                    nc.scalar.copy(out=w_nxt, in_=php)
                    w_cur = w_nxt
                    pv_cur = pv_nxt

            # --- c = w^T phi_last' ; broadcast to O partitions; scale output ---
            cps = phpool.tile([1, 1], dt.float32, name="php")
            nc.tensor.matmul(out=cps, lhsT=plc_sb, rhs=w_cur, start=True, stop=True)
            c_sb = mpool.tile([1, 1], dt.float32, name="c_sb")
            nc.scalar.copy(out=c_sb, in_=cps)
            c10_sb = mpool.tile([O, 1], dt.float32, name="c10_sb")
            nc.gpsimd.partition_broadcast(c10_sb, c_sb)

            out_sb = mpool.tile([O, BL], dt.float32, name="out_sb")
            for t in range(NT):
                nc.scalar.mul(out=out_sb[:, t * TN:(t + 1) * TN], in_=po[t], mul=c10_sb)
            nc.sync.dma_start(out=out_d, in_=out_sb)

    nc.compile()
    return nc


def _scale_schedule(x, psi_first, psi_mid, nsub=128):
    """Static per-step power-of-2 downscales keeping |v| in a small band."""
    xs = np.asarray(x[:nsub], np.float32)
    v = xs[:, 0] @ np.asarray(psi_first, np.float32).T
    ks = []
    for i in range(NMID):
        A = np.asarray(psi_mid[i], np.float32)            # [d, e, p]
        xi = xs[:, 1 + i]                                  # [s, p]
        u = np.einsum('sd,sp->sdp', v, xi).reshape(nsub, D * P)
        v = u @ A.transpose(0, 2, 1).reshape(D * P, D)
        vm = float(np.abs(v).max())
        k = 0
        while vm * 2.0 ** (-k) > VBAND:
            k += 1
        ks.append(k)
        v = v * 2.0 ** (-k)
    return ks


def kernel(x, psi_first, psi_mid, psi_last, phi_first, phi_mid, phi_last):
    _ensure_path()
    from concourse import bass_utils

    f16 = np.float16
    x = np.asarray(x, np.float32)
    psi_first = np.asarray(psi_first, np.float32)
    psi_mid = np.asarray(psi_mid, np.float32)
    psi_last = np.asarray(psi_last, np.float32)
    phi_first = np.asarray(phi_first, np.float32)
    phi_mid = np.asarray(phi_mid, np.float32)
    phi_last = np.asarray(phi_last, np.float32)

    if "nc" not in _cached:
        _cached["nc"] = _build_program()
    nc = _cached["nc"]

    ks = _scale_schedule(x, psi_first, psi_mid)

    # --- shared weight-side arrays (p-major rows, duplicated columns) ---
    scales = (2.0 ** -np.asarray(ks, np.float64)).astype(np.float32)
    # A2[i, r, e] = psi_mid[i, d(r), e, p(r)] * s_i  -> [62, 2048, 64]
    A2 = psi_mid.transpose(0, 1, 3, 2)[:, _D_IDX, _P_IDX, :]        # [62, 2048, 64]
    A2 = A2 * scales[:, None, None]
    A2c = A2.reshape(NMID, NCH, 128, D)
    A2dup = np.concatenate([A2c, A2c], axis=3)                      # [62, 16, 128, 128]
    a_host = np.ascontiguousarray(
        A2dup.transpose(0, 2, 1, 3).reshape(NMID, 128, NCH * 128)
    ).astype(f16)

    pf_host = np.concatenate([psi_first.T, psi_first.T], axis=1).astype(f16)  # [32, 128]

    # pl2[r, o] = psi_last[d(r), p(r), o] * 2^SH -> chunked [128, 16*O]
    pl2 = (psi_last * (2.0 ** SH_LAST))[_D_IDX, _P_IDX, :]          # [2048, O]
    pl_host = np.ascontiguousarray(
        pl2.reshape(NCH, 128, O).transpose(1, 0, 2).reshape(128, NCH * O)
    ).astype(f16)

    phiM = phi_mid[np.arange(NMID), :, :, np.arange(1, NMID + 1)]   # [62, e, f]
    pm_host = np.ascontiguousarray(phiM.transpose(1, 0, 2).reshape(D, NMID * D)).astype(np.float32)
    w0_host = np.ascontiguousarray(phi_first[:, 0:1]).astype(np.float32)
    plc_host = np.ascontiguousarray(
        phi_last[:, Q - 1:Q] * (2.0 ** (sum(ks) - SH_LAST))
    ).astype(np.float32)

    # --- per-core batch shards: x-side broadcast [63, NG, 128, 4*BL] ---
    xt = x.transpose(1, 2, 0).astype(f16)         # [Q, P, B]
    x0_all = xt[0]                                # [P, B]
    # chunk c rows r(128): p = 2c + r//64 ; group tile [128, 4*BL]:
    #   xb[i, g, r, q*BL + s] = xt[1+i, 2*(4g+q) + r//64, s]
    in_maps = []
    for ci in range(N_CORES):
        sl = slice(ci * BL, (ci + 1) * BL)
        xs = np.ascontiguousarray(xt[1:, :, sl])            # [63, P, BL]
        xb = xs[:, np.repeat(np.arange(P), D), :]           # [63, 2048, BL]
        xb = xb.reshape(NMID + 1, NG, 2, 128, BL).transpose(0, 1, 3, 2, 4)
        xb = np.ascontiguousarray(xb.reshape(NMID + 1, NG, 128, 2 * BL))
        in_maps.append({
            "a_w": a_host,
            "xb": xb,
            "x0": np.ascontiguousarray(x0_all[:, sl]),
            "pf": pf_host,
            "pl": pl_host,
            "phim": pm_host,
            "w0": w0_host,
            "phil": plc_host,
        })

    res = bass_utils.run_bass_kernel_spmd(nc, in_maps, core_ids=list(range(N_CORES)))
    _cached["in_maps"] = in_maps

    out = np.empty((B, O), np.float32)
    for ci in range(N_CORES):
        out[ci * BL:(ci + 1) * BL, :] = res.results[ci]["out"].T
    return out


# revision 15
# speedup vs baseline: 1.3803x; 1.0041x over previous
"""Trainium2 Bass kernel for the CMPO2/GTN MPS-chain contraction.

Computation (see harness reference): for each sample s,
    v0  = psi_first^T x[s,0]                                  [D]
    v_{i+1}[e] = sum_{d,p} v_i[d] psi_mid[i][d,e,p] x[s,1+i,p]   (62 steps)
    out_vec[s] = sum_{d,p} v[d] psi_last[d,p,:] x[s,63,p]     [O]
    out[s] = c * out_vec[s]   with c the (batch-independent) phi-chain scalar.

Strategy: data-parallel over batch across 8 cores (1024 samples/core),
MPS parameters replicated.  Per middle step the contraction is mapped as
    u[s,(p,d)] = v[s,d] * x[s,p]        (outer product, fp16, p-major rows)
    v_new      = u @ A_flat             (PE matmul, K=2048 in 16 chunks of 128)
The per-sample v broadcast is obtained for free: the A stationaries are
column-duplicated so each accumulation chain outputs [vT; vT] on all 128
PSUM partitions, and the x-side partition broadcast (which is input data,
not dependent on v) is precomputed on the host and streamed from HBM by
the otherwise-idle DMA engines.  The vector engine does the outer products
(fp16 2x mode); the scalar engine only evacuates the small [vT; vT].
fp16 overflow is prevented by folding static power-of-2 scales (derived
from a host-side subsample) into the A weights; the inverse scale is
folded into the phi-chain scalar, computed on-device in fp32.
"""

import numpy as np

N_CORES = 8
B, Q, P, D, L, O = 8192, 64, 32, 64, 64, 10
BL = B // N_CORES          # batch per core
TN = 512                   # matmul free-dim tile (one PSUM bank of fp32)
NT = BL // TN              # N tiles per batch shard
NCH = (D * P) // 128       # 16 K-chunks of 128 over (p,d)
NG = 8                     # chunk pairs (2 chunks each) for paired DVE muls
NMID = L - 2               # 62 middle sites
SH_LAST = 6                # 2^SH_LAST folded into psi_last (fp16 subnormal avoidance)
VBAND = 16.0               # target |v| band for the scale schedule

# global row r in 0..2047 of u/(A rows): p = 2*(r//128) + (r%128)//64 ; d = r%64
_P_IDX = np.repeat(np.arange(P), D)          # [2048]
_D_IDX = np.tile(np.arange(D), P)            # [2048]

_cached = {}


def _ensure_path():
    import sys
    for p in ("/opt/trn_rl_repo", "/root/.axon_site/_ro/trn_rl_repo"):
        try:
            import concourse  # noqa: F401
            return
        except Exception:
            if p not in sys.path:
                sys.path.insert(0, p)
    import concourse  # noqa: F401


def _build_program():
    """Build + compile the Bass/Tile program (shared by all 8 cores)."""
    _ensure_path()
    from concourse import bacc, tile, mybir

    dt = mybir.dt
    nc = bacc.Bacc(
        "TRN2",
        target_bir_lowering=False,
        debug=False,
        enable_asserts=False,
        num_devices=N_CORES,
    )

    a_d = nc.dram_tensor("a_w", [NMID, 128, NCH * D], dt.float16, kind="ExternalInput").ap()
    xb_d = nc.dram_tensor("xb", [NMID + 1, NG, 128, 2 * BL], dt.float16, kind="ExternalInput").ap()
    x0_d = nc.dram_tensor("x0", [P, BL], dt.float16, kind="ExternalInput").ap()
    pf_d = nc.dram_tensor("pf", [P, 128], dt.float16, kind="ExternalInput").ap()
    pl_d = nc.dram_tensor("pl", [128, NCH * O], dt.float16, kind="ExternalInput").ap()
    pm_d = nc.dram_tensor("phim", [D, NMID * D], dt.float32, kind="ExternalInput").ap()
    w0_d = nc.dram_tensor("w0", [D, 1], dt.float32, kind="ExternalInput").ap()
    plc_d = nc.dram_tensor("phil", [D, 1], dt.float32, kind="ExternalInput").ap()
    out_d = nc.dram_tensor("out", [O, BL], dt.float32, kind="ExternalOutput").ap()

    with tile.TileContext(nc) as tc:
        with tc.tile_pool(name="const", bufs=1) as cpool, \
             tc.tile_pool(name="aw", bufs=3) as apool, \
             tc.tile_pool(name="awd", bufs=2) as adpool, \
             tc.tile_pool(name="xbp", bufs=20) as xbpool, \
             tc.tile_pool(name="vrp", bufs=2) as vrpool, \
             tc.tile_pool(name="up", bufs=10) as upool, \
             tc.tile_pool(name="misc", bufs=1) as mpool, \
             tc.tile_pool(name="wp", bufs=2) as wpool, \
             tc.tile_pool(name="pvp", bufs=4, space="PSUM") as pvpool, \
             tc.tile_pool(name="phpp", bufs=2, space="PSUM") as phpool:

            # --- constants / per-core inputs resident in SBUF ---
            pf_sb = cpool.tile([P, 128], dt.float16, name="pf_sb")
            nc.sync.dma_start(out=pf_sb, in_=pf_d)
            pl_sb = cpool.tile([128, NCH * O], dt.float16, name="pl_sb")
            nc.sync.dma_start(out=pl_sb, in_=pl_d)
            pm_sb = cpool.tile([D, NMID * D], dt.float32, name="pm_sb")
            nc.sync.dma_start(out=pm_sb, in_=pm_d)
            plc_sb = cpool.tile([D, 1], dt.float32, name="plc_sb")
            nc.sync.dma_start(out=plc_sb, in_=plc_d)
            x0_sb = cpool.tile([P, BL], dt.float16, name="x0_sb")
            nc.sync.dma_start(out=x0_sb, in_=x0_d)

            w_cur = wpool.tile([D, 1], dt.float32, name="wv")
            nc.sync.dma_start(out=w_cur, in_=w0_d)

            # --- v0 = [psi_first^T | psi_first^T] @ x0 -> [v0 ; v0] ---
            pv_cur = []
            for t in range(NT):
                pv = pvpool.tile([128, TN], dt.float32, name="pv")
                nc.tensor.matmul(out=pv, lhsT=pf_sb,
                                 rhs=x0_sb[:, t * TN:(t + 1) * TN],
                                 start=True, stop=True)
                pv_cur.append(pv)

            po = None
            for i in range(NMID + 1):
                last = (i == NMID)
                # evacuate [vT; vT] into v2[:, 0:BL] per N-tile half, each
                # followed by its own dup copy, so each half's outer products
                # and matmuls can proceed while the other half is still in
                # flight (keeps the PE warm and busy across the step chain).
                v2 = vrpool.tile([128, 2 * BL], dt.float16, name="v2")
                for t in range(NT):
                    nc.scalar.copy(out=v2[:, t * TN:(t + 1) * TN], in_=pv_cur[t])
                    nc.vector.tensor_copy(v2[:, BL + t * TN:BL + (t + 1) * TN],
                                          v2[:, t * TN:(t + 1) * TN])

                # x-side broadcast tiles from HBM (prefetched by Tile)
                xbs = []
                for g in range(NG):
                    xb_sb = xbpool.tile([128, 2 * BL], dt.float16, name="xb_sb")
                    eng = nc.sync if g % 2 == 0 else nc.scalar
                    eng.dma_start(out=xb_sb, in_=xb_d[i, g])
                    xbs.append(xb_sb)

                if not last:
                    # stream the un-duplicated A chunk-columns, duplicate
                    # on-chip (two strided copies on the idle scalar engine)
                    a_raw = apool.tile([128, NCH * D], dt.float16, name="a_raw")
                    nc.gpsimd.dma_start(out=a_raw, in_=a_d[i])
                    a_sb = adpool.tile([128, NCH * 128], dt.float16, name="a_sb")
                    av = a_sb.rearrange("p (c j e) -> p c j e", c=NCH, j=2, e=D)
                    ar = a_raw.rearrange("p (c e) -> p c e", c=NCH)
                    nc.scalar.copy(out=av[:, :, 0, :], in_=ar)
                    nc.scalar.copy(out=av[:, :, 1, :], in_=ar)
                    pv_nxt = []
                    for t in range(NT):
                        pv = pvpool.tile([128, TN], dt.float32, name="pv")
                        pv_nxt.append(pv)
                else:
                    po = []
                    for t in range(NT):
                        p_o = pvpool.tile([O, TN], dt.float32, name="pv")
                        po.append(p_o)

                # outer products and matmuls, emitted per N-tile half so the
                # two halves software-pipeline: while the DVE produces half
                # t1's u tiles, the PE consumes half t0's.  Columns of
                # v2/xb/u tiles are laid out (q, t, s) with q the chunk
                # within the pair, so the t-half of a pair is a strided view.
                us = []
                for g in range(NG):
                    u2 = upool.tile([128, 2 * BL], dt.float16, name="u2")
                    us.append(u2)
                lhs_sb = pl_sb if last else a_sb
                lhs_w = O if last else 128
                out_ps = po if last else pv_nxt
                for t in range(NT):
                    for q in range(2):
                        sl = slice(q * BL + t * TN, q * BL + (t + 1) * TN)
                        nc.vector.tensor_mul(us[0][:, sl], v2[:, t * TN:(t + 1) * TN],
                                             xbs[0][:, sl])
                    for g in range(1, NG):
                        v2t = v2.rearrange("p (q t s) -> p q t s", q=2, t=NT, s=TN)[:, :, t, :]
                        xbt = xbs[g].rearrange("p (q t s) -> p q t s", q=2, t=NT, s=TN)[:, :, t, :]
                        ut = us[g].rearrange("p (q t s) -> p q t s", q=2, t=NT, s=TN)[:, :, t, :]
                        nc.vector.tensor_mul(ut, v2t, xbt)
                    for c in range(NCH):
                        g, q = c // 2, c % 2
                        sl = slice(q * BL + t * TN, q * BL + (t + 1) * TN)
                        nc.tensor.matmul(
                            out=out_ps[t],
                            lhsT=lhs_sb[:, c * lhs_w:(c + 1) * lhs_w],
                            rhs=us[g][:, sl],
                            start=(c == 0), stop=(c == NCH - 1))
                if not last:
                    # phi chain matvec, interleaved (PE fp32, tiny)
                    php = phpool.tile([D, 1], dt.float32, name="php")
                    nc.tensor.matmul(out=php, lhsT=pm_sb[:, i * D:(i + 1) * D],
                                     rhs=w_cur, start=True, stop=True)
                    w_nxt = wpool.tile([D, 1], dt.float32, name="wv")
                    nc.scalar.copy(out=w_nxt, in_=php)
                    w_cur = w_nxt
                    pv_cur = pv_nxt

            # --- c = w^T phi_last' ; broadcast to O partitions; scale output ---
            cps = phpool.tile([1, 1], dt.float32, name="php")
            nc.tensor.matmul(out=cps, lhsT=plc_sb, rhs=w_cur, start=True, stop=True)
            c_sb = mpool.tile([1, 1], dt.float32, name="c_sb")
            nc.scalar.copy(out=c_sb, in_=cps)
            c10_sb = mpool.tile([O, 1], dt.float32, name="c10_sb")
            nc.gpsimd.partition_broadcast(c10_sb, c_sb)

            out_sb = mpool.tile([O, BL], dt.float32, name="out_sb")
            for t in range(NT):
                nc.scalar.mul(out=out_sb[:, t * TN:(t + 1) * TN], in_=po[t], mul=c10_sb)
            nc.sync.dma_start(out=out_d, in_=out_sb)

    nc.compile()
    return nc


def _scale_schedule(x, psi_first, psi_mid, nsub=128):
    """Static per-step power-of-2 downscales keeping |v| in a small band."""
    xs = np.asarray(x[:nsub], np.float32)
    v = xs[:, 0] @ np.asarray(psi_first, np.float32).T
    ks = []
    for i in range(NMID):
        A = np.asarray(psi_mid[i], np.float32)            # [d, e, p]
        xi = xs[:, 1 + i]                                  # [s, p]
        u = np.einsum('sd,sp->sdp', v, xi).reshape(nsub, D * P)
        v = u @ A.transpose(0, 2, 1).reshape(D * P, D)
        vm = float(np.abs(v).max())
        k = 0
        while vm * 2.0 ** (-k) > VBAND:
            k += 1
        ks.append(k)
        v = v * 2.0 ** (-k)
    return ks


def kernel(x, psi_first, psi_mid, psi_last, phi_first, phi_mid, phi_last):
    _ensure_path()
    from concourse import bass_utils

    f16 = np.float16
    x = np.asarray(x, np.float32)
    psi_first = np.asarray(psi_first, np.float32)
    psi_mid = np.asarray(psi_mid, np.float32)
    psi_last = np.asarray(psi_last, np.float32)
    phi_first = np.asarray(phi_first, np.float32)
    phi_mid = np.asarray(phi_mid, np.float32)
    phi_last = np.asarray(phi_last, np.float32)

    if "nc" not in _cached:
        _cached["nc"] = _build_program()
    nc = _cached["nc"]

    ks = _scale_schedule(x, psi_first, psi_mid)

    # --- shared weight-side arrays (p-major rows, duplicated columns) ---
    scales = (2.0 ** -np.asarray(ks, np.float64)).astype(np.float32)
    # A2[i, r, e] = psi_mid[i, d(r), e, p(r)] * s_i  -> [62, 2048, 64]
    A2 = psi_mid.transpose(0, 1, 3, 2)[:, _D_IDX, _P_IDX, :]        # [62, 2048, 64]
    A2 = A2 * scales[:, None, None]
    A2c = A2.reshape(NMID, NCH, 128, D)
    a_host = np.ascontiguousarray(
        A2c.transpose(0, 2, 1, 3).reshape(NMID, 128, NCH * D)
    ).astype(f16)

    pf_host = np.concatenate([psi_first.T, psi_first.T], axis=1).astype(f16)  # [32, 128]

    # pl2[r, o] = psi_last[d(r), p(r), o] * 2^SH -> chunked [128, 16*O]
    pl2 = (psi_last * (2.0 ** SH_LAST))[_D_IDX, _P_IDX, :]          # [2048, O]
    pl_host = np.ascontiguousarray(
        pl2.reshape(NCH, 128, O).transpose(1, 0, 2).reshape(128, NCH * O)
    ).astype(f16)

    phiM = phi_mid[np.arange(NMID), :, :, np.arange(1, NMID + 1)]   # [62, e, f]
    pm_host = np.ascontiguousarray(phiM.transpose(1, 0, 2).reshape(D, NMID * D)).astype(np.float32)
    w0_host = np.ascontiguousarray(phi_first[:, 0:1]).astype(np.float32)
    plc_host = np.ascontiguousarray(
        phi_last[:, Q - 1:Q] * (2.0 ** (sum(ks) - SH_LAST))
    ).astype(np.float32)

    # --- per-core batch shards: x-side broadcast [63, NG, 128, 4*BL] ---
    xt = x.transpose(1, 2, 0).astype(f16)         # [Q, P, B]
    x0_all = xt[0]                                # [P, B]
    # chunk c rows r(128): p = 2c + r//64 ; group tile [128, 4*BL]:
    #   xb[i, g, r, q*BL + s] = xt[1+i, 2*(4g+q) + r//64, s]
    in_maps = []
    for ci in range(N_CORES):
        sl = slice(ci * BL, (ci + 1) * BL)
        xs = np.ascontiguousarray(xt[1:, :, sl])            # [63, P, BL]
        xb = xs[:, np.repeat(np.arange(P), D), :]           # [63, 2048, BL]
        xb = xb.reshape(NMID + 1, NG, 2, 128, BL).transpose(0, 1, 3, 2, 4)
        xb = np.ascontiguousarray(xb.reshape(NMID + 1, NG, 128, 2 * BL))
        in_maps.append({
            "a_w": a_host,
            "xb": xb,
            "x0": np.ascontiguousarray(x0_all[:, sl]),
            "pf": pf_host,
            "pl": pl_host,
            "phim": pm_host,
            "w0": w0_host,
            "phil": plc_host,
        })

    res = bass_utils.run_bass_kernel_spmd(nc, in_maps, core_ids=list(range(N_CORES)))
    _cached["in_maps"] = in_maps

    out = np.empty((B, O), np.float32)
    for ci in range(N_CORES):
        out[ci * BL:(ci + 1) * BL, :] = res.results[ci]["out"].T
    return out
